# revision 4
# baseline (speedup 1.0000x reference)
"""Trainium2 Bass kernel for causal multi-head attention with full-dim rotary.

Computes, for inputs q,k,v [B=2, L=2048, D=1024] and weights Wq/Wk/Wv/Wo [D,D]:
    Q = rope(q @ Wq.T + bq); K = rope(k @ Wk.T + bk); V = v @ Wv.T + bv
    out = softmax_causal(Qh Kh^T / sqrt(dh)) Vh  (H=16 heads, dh=64)
    y = out @ Wo.T + bo
Sharding: 8 cores = (batch b in {0,1}) x (block of 4 heads). Host sums the 4
partial y per batch and adds the bias correction row (bv @ Wo.T + bo).

v2 design notes:
- Entire attention + out-projection phase runs in the (64,128) PE tile grid:
  scores are row-tiled per head (contraction dh=64, tiles T0/T8 concurrent),
  PV is split-contraction (keys 0:64 on T0, 64:128 on T8) accumulating into
  the same PSUM bank, out-proj contraction (128) split the same way. No PE
  mode-switch drains inside the phase.
- Attention walks 512-wide query chunks; scores for 2 key-tiles x 2 heads
  land in one 4-bank PSUM group, consumed by one wide exp ACTIVATE
  (minimizes the ~352-cycle per-instruction scalar tax). PSUM: 4 (scores)
  + 2 (po accumulators) + 2 (out-proj) = 8 banks.
- Out-proj matmuls for chunk c are deferred and interleaved into chunk c+1's
  rounds as PE filler under the exp shadow.
- Softmax normalize: 1/l via DVE reciprocal, broadcast across partitions on
  gpsimd (partition_broadcast), multiply on DVE. No PE involvement.
- Rope: scalar engine evacuates QK PSUM to bf16, DVE does the 4 elementwise
  ops in 2x bf16 mode.
- All DRAM operands pre-arranged host-side for contiguous per-partition DMA.
"""

import sys
import functools
import numpy as np

try:
    import concourse.bass as bass
except ImportError:  # fresh grading dir: concourse lives in the container image
    sys.path.insert(0, "/opt/trn_rl_repo")
    import concourse.bass as bass

import ml_dtypes
import concourse.mybir as mybir
import concourse.tile as tile
from concourse import bacc
from concourse.bass_utils import run_bass_kernel_spmd

BF16 = mybir.dt.bfloat16
F16 = mybir.dt.float16
F32 = mybir.dt.float32
AF = mybir.ActivationFunctionType

B, D, H, DH = 2, 1024, 16, 64
P = 128
KT = D // P            # 8 contraction tiles for the projections
HPC = 4                # heads per core
NPR = 2                # head-pairs per core
N_CORES = 8
MAXPOS = 10000.0
MC = 512               # m-chunk (rows) for the projection phase
CW = 512               # attention query-chunk width

USE_GPS_BCAST = False  # gpsimd partition_broadcast for 1/l (else PE ones-matmul)


def build_program(L, qk_bias=False):
    NT = L // P                       # key tiles
    NCH = L // CW                     # query chunks
    TPC = CW // P                     # key tiles added per chunk (4)

    nc = bacc.Bacc(None)
    xq = nc.declare_dram_parameter("xq", [P, KT, L], BF16, isOutput=False)
    xk = nc.declare_dram_parameter("xk", [P, KT, L], BF16, isOutput=False)
    xv = nc.declare_dram_parameter("xv", [P, KT, L], BF16, isOutput=False)
    wq = nc.declare_dram_parameter("wq", [P, KT, 2 * P], BF16, isOutput=False)
    wk = nc.declare_dram_parameter("wk", [P, KT, 2 * P], BF16, isOutput=False)
    wv = nc.declare_dram_parameter("wv", [P, KT, 2 * P], BF16, isOutput=False)
    wo = nc.declare_dram_parameter("wo", [P, NPR, D], BF16, isOutput=False)
    ctab = nc.declare_dram_parameter("ctab", [P, NPR, L], BF16, isOutput=False)
    stab = nc.declare_dram_parameter("stab", [P, NPR, L], BF16, isOutput=False)
    if qk_bias:
        rqt = nc.declare_dram_parameter("rqt", [P, NPR, L], BF16, isOutput=False)
        rkt = nc.declare_dram_parameter("rkt", [P, NPR, L], BF16, isOutput=False)
    tri = nc.declare_dram_parameter("tri", [P, P], BF16, isOutput=False)
    y = nc.declare_dram_parameter("y", [L, D], F16, isOutput=True)

    with tile.TileContext(nc) as tc:
        from contextlib import ExitStack

        with ExitStack() as ctx:
            consts = ctx.enter_context(tc.tile_pool(name="consts", bufs=1))
            qk_sb = ctx.enter_context(tc.tile_pool(name="qk_sb", bufs=1))

            # ---- constants (DMA order: weights first, tables next) ----
            wq_sb = consts.tile([P, KT, 2 * P], BF16, tag="wq")
            wk_sb = consts.tile([P, KT, 2 * P], BF16, tag="wk")
            wv_sb = consts.tile([P, KT, 2 * P], BF16, tag="wv")
            wo_sb = consts.tile([P, NPR, D], BF16, tag="wo")
            c_sb = consts.tile([P, NPR, L], BF16, tag="ctab")
            s_sb = consts.tile([P, NPR, L], BF16, tag="stab")
            tri_sb = consts.tile([P, P], BF16, tag="tri")
            nc.sync.dma_start(wq_sb[:], wq[:])
            nc.sync.dma_start(wk_sb[:], wk[:])
            nc.sync.dma_start(wv_sb[:], wv[:])
            nc.sync.dma_start(c_sb[:], ctab[:])
            nc.sync.dma_start(s_sb[:], stab[:])
            nc.sync.dma_start(wo_sb[:], wo[:])
            nc.sync.dma_start(tri_sb[:], tri[:])
            if qk_bias:
                rq_sb = consts.tile([P, NPR, L], BF16, tag="rqt")
                rk_sb = consts.tile([P, NPR, L], BF16, tag="rkt")
                nc.sync.dma_start(rq_sb[:], rqt[:])
                nc.sync.dma_start(rk_sb[:], rkt[:])
            else:
                rq_sb = rk_sb = None
            ones64 = consts.tile([1, DH], BF16, tag="ones64")
            nc.vector.memset(ones64[:], 1.0)
            # force the Exp table-set load into the phase-1 shadow
            dm0 = consts.tile([1, 8], F32, tag="dm0")
            dm1 = consts.tile([1, 8], F32, tag="dm1")
            nc.vector.memset(dm0[:], 0.0)
            nc.scalar.activation(dm1[:], dm0[:], AF.Exp)

            # persistent activations
            QT = [qk_sb.tile([P, L], BF16, tag=f"QT{pr}", name=f"QT{pr}")
                  for pr in range(NPR)]
            KTt = [qk_sb.tile([P, L], BF16, tag=f"KT{pr}", name=f"KT{pr}")
                   for pr in range(NPR)]
            Vp = qk_sb.tile([P, NT, HPC, DH + 1], BF16, tag="Vp")
            OT = [qk_sb.tile([P, L], BF16, tag=f"OT{pr}", name=f"OT{pr}")
                  for pr in range(NPR)]
            nc.vector.memset(Vp[:, :, :, DH : DH + 1], 1.0)

            # a<->b half swap within each 32-partition quadrant
            SWAP = [(i + 16) % 32 for i in range(32)]

            # ================= Phase 1: projections + rope =================
            with (
                tc.tile_pool(name="xin", bufs=2) as xin,
                tc.tile_pool(name="ph1", bufs=3) as ph1,
                tc.tile_pool(name="pp", bufs=4, space="PSUM") as pp,
            ):
                for m in range(L // MC):
                    ms = m * MC
                    xq_t = xin.tile([P, KT, MC], BF16, tag="xq")
                    xk_t = xin.tile([P, KT, MC], BF16, tag="xk")
                    xv_t = xin.tile([P, KT, MC], BF16, tag="xv")
                    nc.sync.dma_start(xq_t[:], xq[:, :, ms : ms + MC])
                    nc.sync.dma_start(xk_t[:], xk[:, :, ms : ms + MC])
                    nc.sync.dma_start(xv_t[:], xv[:, :, ms : ms + MC])

                    for pr in range(NPR):
                        for w_sb, x_t, r_sb, dst in (
                            (wq_sb, xq_t, rq_sb, QT),
                            (wk_sb, xk_t, rk_sb, KTt),
                        ):
                            ps = pp.tile([P, MC], F32, tag="ps_qk")
                            for kt in range(KT):
                                nc.tensor.matmul(
                                    ps[:],
                                    lhsT=w_sb[:, kt, pr * P : pr * P + P],
                                    rhs=x_t[:, kt, :],
                                    start=(kt == 0),
                                    stop=(kt == KT - 1),
                                )
                            # scalar evacuates PSUM -> bf16 so rope runs 2x DVE
                            qraw = ph1.tile([P, MC], BF16, tag="qraw")
                            nc.scalar.copy(qraw[:], ps[:])
                            t1 = ph1.tile([P, MC], BF16, tag="t1")
                            t2 = ph1.tile([P, MC], BF16, tag="t2")
                            t2s = ph1.tile([P, MC], BF16, tag="t2s")
                            nc.vector.tensor_mul(
                                t1[:], qraw[:], c_sb[:, pr, ms : ms + MC]
                            )
                            nc.vector.tensor_mul(
                                t2[:], qraw[:], s_sb[:, pr, ms : ms + MC]
                            )
                            nc.vector.stream_shuffle(t2s[:], t2[:], SWAP)
                            if qk_bias:
                                t3 = ph1.tile([P, MC], BF16, tag="t3")
                                nc.vector.tensor_add(t3[:], t1[:], t2s[:])
                                nc.vector.tensor_add(
                                    dst[pr][:, ms : ms + MC], t3[:],
                                    r_sb[:, pr, ms : ms + MC],
                                )
                            else:
                                nc.vector.tensor_add(
                                    dst[pr][:, ms : ms + MC], t1[:], t2s[:]
                                )

                    # V projection: row-major [m, n]
                    for msub in range(MC // P):
                        ps_v = pp.tile([P, 2 * P], F32, tag="ps_v")
                        for kt in range(KT):
                            nc.tensor.matmul(
                                ps_v[:],
                                lhsT=xv_t[:, kt, msub * P : msub * P + P],
                                rhs=wv_sb[:, kt, :],
                                start=(kt == 0),
                                stop=(kt == KT - 1),
                            )
                        jt = m * (MC // P) + msub
                        nc.vector.tensor_copy(
                            Vp[:, jt, :, 0:DH],
                            ps_v[:].rearrange("p (h d) -> p h d", d=DH),
                        )

            # ========== Phase 2+3: attention + interleaved out-proj =====
            with (
                tc.tile_pool(name="ps_sg", bufs=1, space="PSUM") as sg_pool,
                tc.tile_pool(name="ps_o", bufs=1, space="PSUM") as po_pool,
                tc.tile_pool(name="ps_y", bufs=1, space="PSUM") as yps_pool,
                tc.tile_pool(name="pts", bufs=2) as pts,
                tc.tile_pool(name="ysb", bufs=2) as ysb,
                tc.tile_pool(name="small", bufs=2) as small,
            ):

                def emit_pv(po, pr, c, pt, t0, t1, last_t):
                    """PV for rounds [t0, t1): split-contraction on T0/T8."""
                    for t in range(t0, t1):
                        i = t - t0
                        rs = max(P * t, CW * c)
                        w = CW * (c + 1) - rs
                        off = rs - CW * c
                        hp = [2 * pr + h2 for h2 in range(2)]
                        # wave A: T0 does h0 keys-lo, T8 does h1 keys-hi
                        nc.tensor.matmul(
                            po[0][0 : DH + 1, off : off + w],
                            lhsT=Vp[0:64, t, hp[0], :],
                            rhs=pt[0:64, i, 0, 0:w],
                            start=(t == 0), stop=False,
                            skip_group_check=True,
                        )
                        nc.tensor.matmul(
                            po[1][0 : DH + 1, off : off + w],
                            lhsT=Vp[64:128, t, hp[1], :],
                            rhs=pt[64:128, i, 1, 0:w],
                            start=(t == 0), stop=False,
                            skip_group_check=True,
                        )
                        # wave B: T0 does h1 keys-lo, T8 does h0 keys-hi
                        nc.tensor.matmul(
                            po[1][0 : DH + 1, off : off + w],
                            lhsT=Vp[0:64, t, hp[1], :],
                            rhs=pt[0:64, i, 1, 0:w],
                            start=False, stop=(t == last_t),
                            skip_group_check=True,
                        )
                        nc.tensor.matmul(
                            po[0][0 : DH + 1, off : off + w],
                            lhsT=Vp[64:128, t, hp[0], :],
                            rhs=pt[64:128, i, 0, 0:w],
                            start=False, stop=(t == last_t),
                            skip_group_check=True,
                        )

                def normalize(po, pr, c):
                    """OT[:, chunk] = po[0:64] / po[64] (per head)."""
                    for h2 in range(2):
                        l_sb = small.tile([1, CW], F32, tag=f"lsb{h2}")
                        r32 = small.tile([1, CW], F32, tag=f"r32{h2}")
                        r16 = small.tile([1, CW], BF16, tag=f"r16{h2}")
                        nc.vector.tensor_copy(l_sb[:], po[h2][DH : DH + 1, :])
                        nc.vector.reciprocal_approx_fast(r32[:], l_sb[:])
                        nc.vector.tensor_copy(r16[:], r32[:])
                        if USE_GPS_BCAST:
                            rb = small.tile([DH, CW], BF16, tag=f"rb{h2}")
                            nc.gpsimd.partition_broadcast(rb[:], r16[:])
                        else:
                            # PE ones-matmul broadcast into po[64:128]
                            nc.tensor.matmul(
                                po[h2][DH : DH + DH, :],
                                lhsT=ones64[0:1, :],
                                rhs=r16[:],
                                start=True, stop=True,
                                skip_group_check=True,
                            )
                            rb = small.tile([DH, CW], F32, tag=f"rb{h2}")
                            nc.vector.tensor_copy(rb[:], po[h2][DH : DH + DH, :])
                        nc.vector.tensor_mul(
                            OT[pr][DH * h2 : DH * h2 + DH,
                                   c * CW : (c + 1) * CW],
                            po[h2][0:DH, :],
                            rb[:],
                        )

                def make_outproj(c):
                    """Out-proj emitter units for query rows of chunk c."""
                    units = []
                    for it in range(c * (CW // P), (c + 1) * (CW // P)):
                        def emit(it=it):
                            yt = yps_pool.tile([P, 2, 512], F32, tag="y")
                            for pr2 in range(NPR):
                                for wv2 in range(2):
                                    # T0: (nc2=wv2, kl=0); T8: (nc2=1-wv2, kl=64)
                                    first = (pr2 == 0 and wv2 == 0)
                                    last = (pr2 == NPR - 1 and wv2 == 1)
                                    nc.tensor.matmul(
                                        yt[:, wv2, :],
                                        lhsT=OT[pr2][0:64, it * P : it * P + P],
                                        rhs=wo_sb[0:64, pr2,
                                                  wv2 * 512 : wv2 * 512 + 512],
                                        start=first, stop=last,
                                        skip_group_check=True,
                                    )
                                    nc.tensor.matmul(
                                        yt[:, 1 - wv2, :],
                                        lhsT=OT[pr2][64:128, it * P : it * P + P],
                                        rhs=wo_sb[64:128, pr2,
                                                  (1 - wv2) * 512 :
                                                  (1 - wv2) * 512 + 512],
                                        start=first, stop=last,
                                        skip_group_check=True,
                                    )
                            y_sb = ysb.tile([P, D], F16, tag="ysb")
                            nc.vector.tensor_copy(
                                y_sb[:], yt[:].rearrange("p a b -> p (a b)")
                            )
                            nc.sync.dma_start(
                                y[it * P : it * P + P, :], y_sb[:]
                            )
                        units.append(emit)
                    return units

                fillers = []  # deferred out-proj units from the previous chunk

                def attn_chunk(pr, c):
                    T_c = TPC * (c + 1)
                    po = [po_pool.tile([P, CW], F32, tag=f"o{h2}",
                                       name=f"o{h2}_{pr}_{c}")
                          for h2 in range(2)]
                    prev = None
                    for g in range((T_c + 1) // 2):
                        t0, t1 = 2 * g, min(2 * g + 2, T_c)
                        sg = sg_pool.tile([P, 2, 2, CW], F32, tag="sg",
                                          name=f"sg_{pr}_{c}_{g}")
                        pt = pts.tile([P, 2, 2, CW], BF16, tag=f"pt{g % 2}",
                                      name=f"pt_{pr}_{c}_{g}")
                        diag = False
                        for t in range(t0, t1):
                            i = t - t0
                            rs = max(P * t, CW * c)
                            w = CW * (c + 1) - rs
                            diag = diag or (rs == P * t)
                            for h2 in range(2):
                                hr = DH * h2
                                nc.tensor.matmul(
                                    sg[:, i, h2, 0:w],
                                    lhsT=KTt[pr][hr : hr + DH,
                                                 P * t : P * t + P],
                                    rhs=QT[pr][hr : hr + DH, rs : rs + w],
                                    start=True, stop=True,
                                )
                        # exp: one wide ACTIVATE when the group is full-width
                        if not diag and t1 - t0 == 2:
                            nc.scalar.activation(
                                pt[:, :, :, :], sg[:, :, :, :], AF.Exp,
                                scale=0.125,
                            )
                        else:
                            for t in range(t0, t1):
                                i = t - t0
                                rs = max(P * t, CW * c)
                                w = CW * (c + 1) - rs
                                nc.scalar.activation(
                                    pt[:, i, :, 0:w], sg[:, i, :, 0:w], AF.Exp,
                                    scale=0.125,
                                )
                        # causal tri-mask on diagonal rounds
                        for t in range(t0, t1):
                            if max(P * t, CW * c) == P * t:
                                i = t - t0
                                for h2 in range(2):
                                    nc.vector.tensor_mul(
                                        pt[:, i, h2, 0:P], pt[:, i, h2, 0:P],
                                        tri_sb[:],
                                    )
                        if prev is not None:
                            emit_pv(po, pr, c, *prev, T_c - 1)
                            if fillers:
                                fillers.pop(0)()
                        prev = (pt, t0, t1)
                    emit_pv(po, pr, c, *prev, T_c - 1)
                    if fillers:
                        fillers.pop(0)()
                    normalize(po, pr, c)

                for c in range(NCH):
                    for pr in range(NPR):
                        attn_chunk(pr, c)
                    fillers.extend(make_outproj(c))
                while fillers:
                    fillers.pop(0)()
    nc.compile()
    return nc


@functools.lru_cache(maxsize=2)
def _get_program(L, qk_bias=False):
    return build_program(L, qk_bias)


def _rope_perm(hloc):
    """Column order (within this core's 256 outputs) for head-local index hloc.

    Row r (0..63) of head h: quadrant q = r//32, i = r%32.
    i < 16  -> even dim of freq 16q+i       (a half)
    i >= 16 -> odd dim  of freq 16q+(i-16)  (b half)
    Returns indices into the head's 64 original dims.
    """
    idx = np.zeros(64, dtype=np.int64)
    for r in range(64):
        q, i = divmod(r, 32)
        if i < 16:
            idx[r] = 2 * (16 * q + i)
        else:
            idx[r] = 2 * (16 * q + (i - 16)) + 1
    return idx


def _to_pkl(a):
    """[D, N] -> [P, KT-or-more, N] with row d at [d % P, d // P]."""
    d0, n = a.shape
    return np.ascontiguousarray(a.reshape(d0 // P, P, n).transpose(1, 0, 2))


def _prep_core_inputs(c, L, q, k, v, Wq, bq, Wk, bk, Wv, bv, Wo, bo):
    b = c // (N_CORES // B)
    hb = HPC * (c % (N_CORES // B))  # first global head on this core
    bf = ml_dtypes.bfloat16

    xq = _to_pkl(np.ascontiguousarray(q[b].T)).astype(bf)
    xk = _to_pkl(np.ascontiguousarray(k[b].T)).astype(bf)
    xv = _to_pkl(np.ascontiguousarray(v[b].T)).astype(bf)

    # permuted row order of Wq/Wk for this core's 4 heads
    rows = np.concatenate(
        [64 * (hb + hl) + _rope_perm(hl) for hl in range(HPC)]
    )
    wq_t = _to_pkl(np.ascontiguousarray(Wq[rows, :].T)).astype(bf)  # [P,KT,256]
    wk_t = _to_pkl(np.ascontiguousarray(Wk[rows, :].T)).astype(bf)
    bq_p = bq[rows].astype(np.float64).reshape(NPR, P)
    bk_p = bk[rows].astype(np.float64).reshape(NPR, P)
    vrows = np.arange(64 * hb, 64 * (hb + HPC))
    wv_t = _to_pkl(np.ascontiguousarray(Wv[vrows, :].T)).astype(bf)
    wo_t = np.ascontiguousarray(
        Wo[:, vrows].T.reshape(NPR, P, D).transpose(1, 0, 2)
    ).astype(bf)                                                    # [P,NPR,D]

    # rope tables in permuted row order; sin negated on b halves
    pos = np.arange(L, dtype=np.float64)
    ct = np.zeros((NPR, P, L), dtype=np.float64)
    st = np.zeros((NPR, P, L), dtype=np.float64)
    for pr in range(NPR):
        for h2 in range(2):
            hg = hb + 2 * pr + h2
            for r in range(64):
                qd, i = divmod(r, 32)
                f = 32 * hg + 16 * qd + (i % 16)
                theta = MAXPOS ** (-f / (D // 2))
                ang = pos * theta
                row = DH * h2 + r
                ct[pr, row] = np.cos(ang)
                st[pr, row] = np.sin(ang) if i < 16 else -np.sin(ang)
    ctab = np.ascontiguousarray(
        ct.astype(np.float32).astype(bf).transpose(1, 0, 2))
    stab = np.ascontiguousarray(
        st.astype(np.float32).astype(bf).transpose(1, 0, 2))

    jj = np.arange(P)
    tri = (jj[None, :] >= jj[:, None]).astype(np.float32).astype(bf)

    im = {
        "xq": xq, "xk": xk, "xv": xv,
        "wq": wq_t, "wk": wk_t, "wv": wv_t, "wo": wo_t,
        "ctab": ctab, "stab": stab, "tri": tri,
    }
    if np.abs(bq).max() > 0 or np.abs(bk).max() > 0:
        def swap16(a):
            a4 = a.reshape(NPR, P // 32, 2, 16, L)
            return a4[:, :, ::-1, :, :].reshape(NPR, P, L)

        rqt = bq_p[:, :, None] * ct + swap16(bq_p[:, :, None] * st)
        rkt = bk_p[:, :, None] * ct + swap16(bk_p[:, :, None] * st)
        im["rqt"] = np.ascontiguousarray(
            rqt.astype(np.float32).astype(bf).transpose(1, 0, 2))
        im["rkt"] = np.ascontiguousarray(
            rkt.astype(np.float32).astype(bf).transpose(1, 0, 2))
    return im


def kernel(q, k, v, Wq, bq, Wk, bk, Wv, bv, Wo, bo):
    q, k, v = (np.asarray(a, dtype=np.float32) for a in (q, k, v))
    Wq, bq, Wk, bk, Wv, bv, Wo, bo = (
        np.asarray(a, dtype=np.float32) for a in (Wq, bq, Wk, bk, Wv, bv, Wo, bo)
    )
    Bq, L, Dq = q.shape
    assert (Bq, Dq) == (B, D)

    qk_bias = bool(np.abs(bq).max() > 0 or np.abs(bk).max() > 0)
    nc = _get_program(L, qk_bias)
    in_maps = [
        _prep_core_inputs(c, L, q, k, v, Wq, bq, Wk, bk, Wv, bv, Wo, bo)
        for c in range(N_CORES)
    ]
    res = run_bass_kernel_spmd(nc, in_maps, core_ids=list(range(N_CORES)))

    corr = (bv @ Wo.T + bo).astype(np.float32)  # folded-out V/O biases
    y = np.zeros((B, L, D), dtype=np.float32)
    cpb = N_CORES // B
    for c in range(N_CORES):
        y[c // cpb] += np.asarray(res.results[c]["y"], dtype=np.float32)
    y += corr[None, None, :]
    return y


# revision 57
# speedup vs baseline: 1.4547x; 1.4547x over previous
"""Trainium2 Bass kernel for causal multi-head attention with full-dim rotary.

Computes, for inputs q,k,v [B=2, L=2048, D=1024] and weights Wq/Wk/Wv/Wo [D,D]:
    Q = rope(q @ Wq.T + bq); K = rope(k @ Wk.T + bk); V = v @ Wv.T + bv
    out = softmax_causal(Qh Kh^T / sqrt(dh)) Vh  (H=16 heads, dh=64)
    y = out @ Wo.T + bo
Sharding: 8 cores = (batch b in {0,1}) x (block of 4 heads). Host sums the 4
partial y per batch and adds the bias correction row (bv @ Wo.T + bo).

v2 design notes:
- Entire attention + out-projection phase runs in the (64,128) PE tile grid:
  scores are row-tiled per head (contraction dh=64, tiles T0/T8 concurrent),
  PV is split-contraction (keys 0:64 on T0, 64:128 on T8) accumulating into
  the same PSUM bank, out-proj contraction (128) split the same way. No PE
  mode-switch drains inside the phase.
- Attention walks 512-wide query chunks; scores for 2 key-tiles x 2 heads
  land in one 4-bank PSUM group, consumed by one wide exp ACTIVATE
  (minimizes the ~352-cycle per-instruction scalar tax). PSUM: 4 (scores)
  + 2 (po accumulators) + 2 (out-proj) = 8 banks.
- Out-proj matmuls for chunk c are deferred and interleaved into chunk c+1's
  rounds as PE filler under the exp shadow.
- Softmax normalize: 1/l via DVE reciprocal, broadcast across partitions on
  gpsimd (partition_broadcast), multiply on DVE. No PE involvement.
- Rope: scalar engine evacuates QK PSUM to bf16, DVE does the 4 elementwise
  ops in 2x bf16 mode.
- All DRAM operands pre-arranged host-side for contiguous per-partition DMA.
"""

import sys
import functools
import numpy as np

try:
    import concourse.bass as bass
except ImportError:  # fresh grading dir: concourse lives in the container image
    sys.path.insert(0, "/opt/trn_rl_repo")
    import concourse.bass as bass

import ml_dtypes
import concourse.mybir as mybir
import concourse.tile as tile
from concourse import bacc
from concourse.bass_utils import run_bass_kernel_spmd

BF16 = mybir.dt.bfloat16
F16 = mybir.dt.float16
F32 = mybir.dt.float32
F8 = mybir.dt.float8e4
AF = mybir.ActivationFunctionType

B, D, H, DH = 2, 1024, 16, 64
P = 128
KT = D // P            # 8 contraction tiles for the projections
KT2 = D // 256         # 4 double-row contraction tiles (fp8)
HPC = 4                # heads per core
NPR = 2                # head-pairs per core
N_CORES = 8
MAXPOS = 10000.0
MC = 512               # m-chunk (rows) for the projection phase
CW = 512               # attention query-chunk width

USE_GPS_BCAST = True  # gpsimd partition_broadcast for 1/l (else PE ones-matmul)
FP8_PROJ = False        # fp8e4m3 DoubleRow projections (x, Wq/Wk/Wv in fp8;
                       # weights x8 host-side, 1/8 folded into rope tables
                       # and the V evacuation)
WSCALE = 8.0


def build_program(L, qk_bias=False):
    NT = L // P                       # key tiles
    NCH = L // CW                     # query chunks
    TPC = CW // P                     # key tiles added per chunk (4)

    nc = bacc.Bacc(None)
    if FP8_PROJ:
        XSH, WSH, XDT = [P, KT2, 2, L], [P, KT2, 2, 2 * P], F8
    else:
        XSH, WSH, XDT = [P, KT, L], [P, KT, 2 * P], BF16
    xq = nc.declare_dram_parameter("xq", XSH, XDT, isOutput=False)
    xk = nc.declare_dram_parameter("xk", XSH, XDT, isOutput=False)
    xv = nc.declare_dram_parameter("xv", XSH, XDT, isOutput=False)
    wq = nc.declare_dram_parameter("wq", WSH, XDT, isOutput=False)
    wk = nc.declare_dram_parameter("wk", WSH, XDT, isOutput=False)
    wv = nc.declare_dram_parameter("wv", WSH, XDT, isOutput=False)
    wo = nc.declare_dram_parameter("wo", [P, NPR, D], BF16, isOutput=False)
    ctab = nc.declare_dram_parameter("ctab", [P, NPR, L], BF16, isOutput=False)
    stab = nc.declare_dram_parameter("stab", [P, NPR, L], BF16, isOutput=False)
    if qk_bias:
        rqt = nc.declare_dram_parameter("rqt", [P, NPR, L], BF16, isOutput=False)
        rkt = nc.declare_dram_parameter("rkt", [P, NPR, L], BF16, isOutput=False)
    tri = nc.declare_dram_parameter("tri", [P, P], BF16, isOutput=False)
    y = nc.declare_dram_parameter("y", [L, D], F16, isOutput=True)

    with tile.TileContext(nc) as tc:
        from contextlib import ExitStack

        with ExitStack() as ctx:
            consts = ctx.enter_context(tc.tile_pool(name="consts", bufs=1))
            qk_sb = ctx.enter_context(tc.tile_pool(name="qk_sb", bufs=1))

            # ---- constants (DMA order: weights first, tables next) ----
            wq_sb = consts.tile(WSH, XDT, tag="wq")
            wk_sb = consts.tile(WSH, XDT, tag="wk")
            wv_sb = consts.tile(WSH, XDT, tag="wv")
            wo_sb = consts.tile([P, NPR, D], BF16, tag="wo")
            c_sb = consts.tile([P, NPR, L], BF16, tag="ctab")
            s_sb = consts.tile([P, NPR, L], BF16, tag="stab")
            tri_sb = consts.tile([P, P], BF16, tag="tri")
            # wq first; everything else staged between the first input
            # chunks so the first projection matmul starts ASAP
            nc.sync.dma_start(wq_sb[:], wq[:])
            if qk_bias:
                rq_sb = consts.tile([P, NPR, L], BF16, tag="rqt")
                rk_sb = consts.tile([P, NPR, L], BF16, tag="rkt")
            else:
                rq_sb = rk_sb = None

            def _late_consts(m):
                if m == 0:
                    nc.sync.dma_start(c_sb[:], ctab[:])
                    nc.sync.dma_start(s_sb[:], stab[:])
                    if qk_bias:
                        nc.sync.dma_start(rq_sb[:], rqt[:])
                        nc.sync.dma_start(rk_sb[:], rkt[:])
                elif m == 1:
                    nc.sync.dma_start(wo_sb[:], wo[:])
                    nc.sync.dma_start(tri_sb[:], tri[:])
            ones64 = consts.tile([1, DH], BF16, tag="ones64")
            nc.vector.memset(ones64[:], 1.0)
            # force the Exp table-set load into the phase-1 shadow
            dm0 = consts.tile([1, 8], F32, tag="dm0")
            dm1 = consts.tile([1, 8], F32, tag="dm1")
            nc.vector.memset(dm0[:], 0.0)
            nc.scalar.activation(dm1[:], dm0[:], AF.Exp)

            # persistent activations
            QT = [qk_sb.tile([P, L], BF16, tag=f"QT{pr}", name=f"QT{pr}")
                  for pr in range(NPR)]
            KTt = [qk_sb.tile([P, L], BF16, tag=f"KT{pr}", name=f"KT{pr}")
                   for pr in range(NPR)]
            Vp = qk_sb.tile([P, NT, HPC, DH + 1], BF16, tag="Vp")
            OT = [qk_sb.tile([P, L], BF16, tag=f"OT{pr}", name=f"OT{pr}")
                  for pr in range(NPR)]
            nc.vector.memset(Vp[:, :, :, DH : DH + 1], 1.0)

            # a<->b half swap within each 32-partition quadrant
            SWAP = [(i + 16) % 32 for i in range(32)]

            # ================= Phase 1: projections + rope =================
            with (
                tc.tile_pool(name="xin", bufs=2) as xin,
                tc.tile_pool(name="ph1", bufs=3) as ph1,
                tc.tile_pool(name="pp", bufs=4, space="PSUM") as pp,
            ):
                XTSH = [P, KT2, 2, MC] if FP8_PROJ else [P, KT, MC]
                for m in range(L // MC):
                    ms = m * MC
                    xq_t = xin.tile(XTSH, XDT, tag="xq")
                    xk_t = xin.tile(XTSH, XDT, tag="xk")
                    xv_t = xin.tile(XTSH, XDT, tag="xv")
                    def _xs(xp):
                        if FP8_PROJ:
                            return xp[:, :, :, ms : ms + MC]
                        return xp[:, :, ms : ms + MC]

                    nc.sync.dma_start(xq_t[:], _xs(xq))
                    if m == 0:
                        nc.sync.dma_start(wk_sb[:], wk[:])
                    nc.sync.dma_start(xk_t[:], _xs(xk))
                    if m == 0:
                        nc.sync.dma_start(wv_sb[:], wv[:])
                    nc.sync.dma_start(xv_t[:], _xs(xv))
                    _late_consts(m)

                    for pr in range(NPR):
                        for w_sb, x_t, r_sb, dst in (
                            (wq_sb, xq_t, rq_sb, QT),
                            (wk_sb, xk_t, rk_sb, KTt),
                        ):
                            ps = pp.tile([P, MC], F32, tag="ps_qk")
                            if FP8_PROJ:
                                for kt in range(KT2):
                                    nc.tensor.matmul(
                                        ps[:],
                                        lhsT=w_sb[:, kt, :,
                                                  pr * P : pr * P + P],
                                        rhs=x_t[:, kt, :, :],
                                        start=(kt == 0),
                                        stop=(kt == KT2 - 1),
                                        perf_mode=
                                        mybir.MatmulPerfMode.DoubleRow,
                                    )
                            else:
                                for kt in range(KT):
                                    nc.tensor.matmul(
                                        ps[:],
                                        lhsT=w_sb[:, kt, pr * P : pr * P + P],
                                        rhs=x_t[:, kt, :],
                                        start=(kt == 0),
                                        stop=(kt == KT - 1),
                                    )
                            # scalar evacuates PSUM -> bf16 so rope runs 2x DVE
                            qraw = ph1.tile([P, MC], BF16, tag="qraw")
                            nc.scalar.copy(qraw[:], ps[:])
                            t1 = ph1.tile([P, MC], BF16, tag="t1")
                            t2 = ph1.tile([P, MC], BF16, tag="t2")
                            t2s = ph1.tile([P, MC], BF16, tag="t2s")
                            nc.vector.tensor_mul(
                                t1[:], qraw[:], c_sb[:, pr, ms : ms + MC]
                            )
                            nc.vector.tensor_mul(
                                t2[:], qraw[:], s_sb[:, pr, ms : ms + MC]
                            )
                            nc.vector.stream_shuffle(t2s[:], t2[:], SWAP)
                            if qk_bias:
                                t3 = ph1.tile([P, MC], BF16, tag="t3")
                                nc.vector.tensor_add(t3[:], t1[:], t2s[:])
                                nc.vector.tensor_add(
                                    dst[pr][:, ms : ms + MC], t3[:],
                                    r_sb[:, pr, ms : ms + MC],
                                )
                            else:
                                nc.vector.tensor_add(
                                    dst[pr][:, ms : ms + MC], t1[:], t2s[:]
                                )

                    # V projection: row-major [m, n]
                    for msub in range(MC // P):
                        ps_v = pp.tile([P, 2 * P], F32, tag="ps_v")
                        if FP8_PROJ:
                            for kt in range(KT2):
                                nc.tensor.matmul(
                                    ps_v[:],
                                    lhsT=xv_t[:, kt, :,
                                              msub * P : msub * P + P],
                                    rhs=wv_sb[:, kt, :, :],
                                    start=(kt == 0),
                                    stop=(kt == KT2 - 1),
                                    perf_mode=mybir.MatmulPerfMode.DoubleRow,
                                )
                        else:
                            for kt in range(KT):
                                nc.tensor.matmul(
                                    ps_v[:],
                                    lhsT=xv_t[:, kt, msub * P : msub * P + P],
                                    rhs=wv_sb[:, kt, :],
                                    start=(kt == 0),
                                    stop=(kt == KT - 1),
                                )
                        jt = m * (MC // P) + msub
                        if FP8_PROJ:
                            # undo the x8 weight scaling during evacuation
                            nc.vector.tensor_scalar_mul(
                                Vp[:, jt, :, 0:DH],
                                ps_v[:].rearrange("p (h d) -> p h d", d=DH),
                                1.0 / WSCALE,
                            )
                        else:
                            nc.vector.tensor_copy(
                                Vp[:, jt, :, 0:DH],
                                ps_v[:].rearrange("p (h d) -> p h d", d=DH),
                            )

            # ========== Phase 2+3: attention + interleaved out-proj =====
            # PSUM: 2 score slots x 2 banks + 4 po banks (2 instance
            # parities x 2 heads) = 8. Out-proj tiles borrow idle-parity po
            # tags so chunk c's out-proj interleaves with later rounds as
            # PE filler under the exp shadow.
            with (
                tc.tile_pool(name="ps_sg", bufs=1, space="PSUM") as sg_pool,
                tc.tile_pool(name="ps_o", bufs=1, space="PSUM") as po_pool,
                tc.tile_pool(name="pts", bufs=1) as pts,
                tc.tile_pool(name="ysb", bufs=2) as ysb,
                tc.tile_pool(name="small", bufs=2) as small,
            ):
                rc = [0]      # global round counter -> sg slot ring
                ptc = [0]     # global pt ring counter
                nin = [0]     # attention instance counter (parity -> po tags)
                PT_RING = 6
                MAX_PEND = 4  # PV lag in rounds (lets po/normalize decouple)

                pend = []     # (po, pr, c, pt, t, last_t) awaiting PV
                npv = [0]     # PVs emitted so far
                nq = []       # normalize steps: (gate, emitter)
                oq = []       # out-proj units: {"gate", "steps", "idx"}

                def emit_pv(po, pr, c, pt, t, last_t):
                    """PV for round t: full-128 contraction per head.

                    (Split-contraction on T0/T8 row-tiles would be ~the same
                    wall time but lets two row tiles hit one PSUM bank in
                    near-overlap, which the HW forbids — observed as a
                    runtime fault.)"""
                    rs = max(P * t, CW * c)
                    w = CW * (c + 1) - rs
                    off = rs - CW * c
                    for h2 in range(2):
                        nc.tensor.matmul(
                            po[h2][0 : DH + 1, off : off + w],
                            lhsT=Vp[:, t, 2 * pr + h2, :],
                            rhs=pt[:, h2, 0:w],
                            start=(t == 0), stop=(t == last_t),
                            skip_group_check=True,
                        )

                def push_normalize(po, pr, c):
                    """OT[:, chunk] = po[0:64] / po[64], as filler lambdas."""
                    hold = {}

                    def recip(h2):
                        # l-copy + SBUF reciprocal (known good); no bf16
                        # cast — gpsimd broadcasts the fp32 result directly
                        l_sb = small.tile([1, CW], F32, tag=f"lsb{h2}")
                        r32 = small.tile([1, CW], F32, tag=f"r32{h2}")
                        nc.vector.tensor_copy(l_sb[:], po[h2][DH : DH + 1, :])
                        nc.vector.reciprocal_approx_fast(r32[:], l_sb[:])
                        hold[f"r16{h2}"] = r32

                    def bcast(h2):
                        r16 = hold[f"r16{h2}"]
                        if USE_GPS_BCAST:
                            rb = small.tile([DH, CW], F32, tag=f"rb{h2}")
                            nc.gpsimd.partition_broadcast(rb[:], r16[:])
                        else:
                            nc.tensor.matmul(
                                po[h2][DH : DH + DH, :],
                                lhsT=ones64[0:1, :],
                                rhs=r16[:],
                                start=True, stop=True,
                                skip_group_check=True,
                            )
                            rb = small.tile([DH, CW], F32, tag=f"rb{h2}")
                            nc.vector.tensor_copy(rb[:], po[h2][DH : DH + DH, :])
                        hold[f"rb{h2}"] = rb

                    def mul(h2):
                        nc.vector.tensor_mul(
                            OT[pr][DH * h2 : DH * h2 + DH,
                                   c * CW : (c + 1) * CW],
                            po[h2][0:DH, :],
                            hold[f"rb{h2}"],
                        )

                    gate = npv[0] + len(pend)  # all PVs of this instance
                    # recips first, then bcasts, then muls: the gpsimd
                    # broadcasts overlap the DVE muls instead of serializing
                    for h2 in range(2):
                        nq.append((gate, lambda h2=h2: recip(h2)))
                    for h2 in range(2):
                        nq.append((gate, lambda h2=h2: bcast(h2)))
                    for h2 in range(2):
                        nq.append((gate, lambda h2=h2: mul(h2)))

                def push_outproj(c):
                    """Out-proj of chunk c's rows, as gated filler units.

                    Each unit's yt borrows the po tags of the parity OPPOSITE
                    to the instance it pops in (chosen at pop time); the
                    nq-empty + rounds_left guards in pop_work keep tag reuse
                    in emission order."""
                    gate = npv[0] + len(pend)
                    for it in range(c * (CW // P), (c + 1) * (CW // P)):
                        hold = {}

                        def mmh(wv2, ypar, it=it, hold=hold):
                            # 2-MM half-steps keep per-round PE work uniform
                            if wv2 == 0:
                                hold["yt"] = [
                                    po_pool.tile([P, CW], F32,
                                                 tag=f"o{2 * ypar + w2}",
                                                 name=f"yt_{it}_{w2}")
                                    for w2 in range(2)
                                ]
                                hold["ysb"] = ysb.tile([P, D], F16, tag="ysb",
                                                       name=f"ysb_{it}")
                            for pr2 in range(NPR):
                                nc.tensor.matmul(
                                    hold["yt"][wv2][:],
                                    lhsT=OT[pr2][:, it * P : it * P + P],
                                    rhs=wo_sb[:, pr2,
                                              wv2 * 512 : wv2 * 512 + 512],
                                    start=(pr2 == 0),
                                    stop=(pr2 == NPR - 1),
                                    skip_group_check=True,
                                )

                        def ycopy(half, it=it, hold=hold):
                            # copy + immediate half-DMA overlaps the other half
                            nc.vector.tensor_copy(
                                hold["ysb"][:, half * 512 : half * 512 + 512],
                                hold["yt"][half][:],
                            )
                            nc.sync.dma_start(
                                y[it * P : it * P + P,
                                  half * 512 : half * 512 + 512],
                                hold["ysb"][:, half * 512 : half * 512 + 512],
                            )

                        oq.append({"gate": gate, "idx": 0,
                                   "steps": [
                                       lambda ypar, f=mmh: f(0, ypar),
                                       lambda ypar, f=ycopy: f(0),
                                       lambda ypar, f=mmh: f(1, ypar),
                                       lambda ypar, f=ycopy: f(1),
                                   ]})

                def pop_work(par, rounds_left, nf=2):
                    """Emit one lagged PV + up to nf filler steps."""
                    if len(pend) > MAX_PEND:
                        emit_pv(*pend.pop(0))
                        npv[0] += 1
                    for _ in range(nf):
                        if nq and npv[0] >= nq[0][0]:
                            nq.pop(0)[1]()
                        elif oq and npv[0] >= oq[0]["gate"] and not nq:
                            u = oq[0]
                            if u["idx"] == 0 and rounds_left < 2:
                                break  # don't start a unit we can't finish
                            u["steps"][u["idx"]](1 - par)
                            u["idx"] += 1
                            if u["idx"] == len(u["steps"]):
                                oq.pop(0)
                        else:
                            break

                def attn_chunk(pr, c):
                    T_c = TPC * (c + 1)
                    par = nin[0] % 2
                    nin[0] += 1
                    po = [po_pool.tile([P, CW], F32, tag=f"o{2 * par + h2}",
                                       name=f"o{h2}_{pr}_{c}")
                          for h2 in range(2)]
                    for t in range(T_c):
                        rs = max(P * t, CW * c)
                        w = CW * (c + 1) - rs
                        slot = rc[0] % 2
                        rc[0] += 1
                        pslot = ptc[0] % PT_RING
                        ptc[0] += 1
                        sg = sg_pool.tile([P, 2, CW], F32, tag=f"sg{slot}",
                                          name=f"sg_{pr}_{c}_{t}")
                        pt = pts.tile([P, 2, CW], BF16, tag=f"pt{pslot}",
                                      name=f"pt_{pr}_{c}_{t}")
                        for h2 in range(2):
                            hr = DH * h2
                            nc.tensor.matmul(
                                sg[:, h2, 0:w],
                                lhsT=KTt[pr][hr : hr + DH, P * t : P * t + P],
                                rhs=QT[pr][hr : hr + DH, rs : rs + w],
                                start=True, stop=True,
                            )
                        nc.scalar.activation(
                            pt[:, :, 0:w], sg[:, :, 0:w], AF.Exp, scale=0.125,
                        )
                        if rs == P * t:  # diagonal round: causal tri-mask
                            for h2 in range(2):
                                nc.vector.tensor_mul(
                                    pt[:, h2, 0:P], pt[:, h2, 0:P], tri_sb[:],
                                )
                        pend.append((po, pr, c, pt, t, T_c - 1))
                        pop_work(par, T_c - 1 - t)
                    push_normalize(po, pr, c)

                for c in range(NCH):
                    for pr in range(NPR):
                        attn_chunk(pr, c)
                    push_outproj(c)
                # tail drain: remaining PVs, then normalize, then out-proj
                while pend:
                    emit_pv(*pend.pop(0))
                    npv[0] += 1
                while nq:
                    nq.pop(0)[1]()
                while oq:
                    u = oq.pop(0)
                    for j in range(u["idx"], len(u["steps"])):
                        u["steps"][j](0)  # borrow parity-0 (pr0) po tags
    nc.compile()
    return nc


@functools.lru_cache(maxsize=2)
def _get_program(L, qk_bias=False):
    return build_program(L, qk_bias)


def _rope_perm(hloc):
    """Column order (within this core's 256 outputs) for head-local index hloc.

    Row r (0..63) of head h: quadrant q = r//32, i = r%32.
    i < 16  -> even dim of freq 16q+i       (a half)
    i >= 16 -> odd dim  of freq 16q+(i-16)  (b half)
    Returns indices into the head's 64 original dims.
    """
    idx = np.zeros(64, dtype=np.int64)
    for r in range(64):
        q, i = divmod(r, 32)
        if i < 16:
            idx[r] = 2 * (16 * q + i)
        else:
            idx[r] = 2 * (16 * q + (i - 16)) + 1
    return idx


def _to_pkl(a):
    """[D, N] -> [P, KT-or-more, N] with row d at [d % P, d // P]."""
    d0, n = a.shape
    return np.ascontiguousarray(a.reshape(d0 // P, P, n).transpose(1, 0, 2))


def _to_dr8(a):
    """[D, N] -> fp8 [P, KT2, 2, N] double-row packing: row d of the
    contraction goes to [ki, kt2, ko, :] with d = 256*kt2 + 2*ki + ko."""
    d0, n = a.shape
    return np.ascontiguousarray(
        a.reshape(d0 // 256, P, 2, n).transpose(1, 0, 2, 3)
    ).astype(ml_dtypes.float8_e4m3)


def _prep_core_inputs(c, L, q, k, v, Wq, bq, Wk, bk, Wv, bv, Wo, bo):
    b = c // (N_CORES // B)
    hb = HPC * (c % (N_CORES // B))  # first global head on this core
    bf = ml_dtypes.bfloat16

    if FP8_PROJ:
        xq = _to_dr8(np.ascontiguousarray(q[b].T))
        xk = _to_dr8(np.ascontiguousarray(k[b].T))
        xv = _to_dr8(np.ascontiguousarray(v[b].T))
    else:
        xq = _to_pkl(np.ascontiguousarray(q[b].T)).astype(bf)
        xk = _to_pkl(np.ascontiguousarray(k[b].T)).astype(bf)
        xv = _to_pkl(np.ascontiguousarray(v[b].T)).astype(bf)

    # permuted row order of Wq/Wk for this core's 4 heads
    rows = np.concatenate(
        [64 * (hb + hl) + _rope_perm(hl) for hl in range(HPC)]
    )
    vrows = np.arange(64 * hb, 64 * (hb + HPC))
    if FP8_PROJ:
        wq_t = _to_dr8(np.ascontiguousarray(Wq[rows, :].T) * WSCALE)
        wk_t = _to_dr8(np.ascontiguousarray(Wk[rows, :].T) * WSCALE)
        wv_t = _to_dr8(np.ascontiguousarray(Wv[vrows, :].T) * WSCALE)
    else:
        wq_t = _to_pkl(np.ascontiguousarray(Wq[rows, :].T)).astype(bf)
        wk_t = _to_pkl(np.ascontiguousarray(Wk[rows, :].T)).astype(bf)
        wv_t = _to_pkl(np.ascontiguousarray(Wv[vrows, :].T)).astype(bf)
    bq_p = bq[rows].astype(np.float64).reshape(NPR, P)
    bk_p = bk[rows].astype(np.float64).reshape(NPR, P)
    wo_t = np.ascontiguousarray(
        Wo[:, vrows].T.reshape(NPR, P, D).transpose(1, 0, 2)
    ).astype(bf)                                                    # [P,NPR,D]

    # rope tables in permuted row order; sin negated on b halves
    pos = np.arange(L, dtype=np.float64)
    ct = np.zeros((NPR, P, L), dtype=np.float64)
    st = np.zeros((NPR, P, L), dtype=np.float64)
    for pr in range(NPR):
        for h2 in range(2):
            hg = hb + 2 * pr + h2
            for r in range(64):
                qd, i = divmod(r, 32)
                f = 32 * hg + 16 * qd + (i % 16)
                theta = MAXPOS ** (-f / (D // 2))
                ang = pos * theta
                row = DH * h2 + r
                ct[pr, row] = np.cos(ang)
                st[pr, row] = np.sin(ang) if i < 16 else -np.sin(ang)
    # the on-device tables also undo the x8 fp8 weight scaling; ct/st stay
    # unscaled for the (rare) rope'd-bias path below
    tsc = 1.0 / WSCALE if FP8_PROJ else 1.0
    ctab = np.ascontiguousarray(
        (ct * tsc).astype(np.float32).astype(bf).transpose(1, 0, 2))
    stab = np.ascontiguousarray(
        (st * tsc).astype(np.float32).astype(bf).transpose(1, 0, 2))

    jj = np.arange(P)
    tri = (jj[None, :] >= jj[:, None]).astype(np.float32).astype(bf)

    im = {
        "xq": xq, "xk": xk, "xv": xv,
        "wq": wq_t, "wk": wk_t, "wv": wv_t, "wo": wo_t,
        "ctab": ctab, "stab": stab, "tri": tri,
    }
    if np.abs(bq).max() > 0 or np.abs(bk).max() > 0:
        def swap16(a):
            a4 = a.reshape(NPR, P // 32, 2, 16, L)
            return a4[:, :, ::-1, :, :].reshape(NPR, P, L)

        rqt = bq_p[:, :, None] * ct + swap16(bq_p[:, :, None] * st)
        rkt = bk_p[:, :, None] * ct + swap16(bk_p[:, :, None] * st)
        im["rqt"] = np.ascontiguousarray(
            rqt.astype(np.float32).astype(bf).transpose(1, 0, 2))
        im["rkt"] = np.ascontiguousarray(
            rkt.astype(np.float32).astype(bf).transpose(1, 0, 2))
    return im


def kernel(q, k, v, Wq, bq, Wk, bk, Wv, bv, Wo, bo):
    q, k, v = (np.asarray(a, dtype=np.float32) for a in (q, k, v))
    Wq, bq, Wk, bk, Wv, bv, Wo, bo = (
        np.asarray(a, dtype=np.float32) for a in (Wq, bq, Wk, bk, Wv, bv, Wo, bo)
    )
    Bq, L, Dq = q.shape
    assert (Bq, Dq) == (B, D)

    qk_bias = bool(np.abs(bq).max() > 0 or np.abs(bk).max() > 0)
    nc = _get_program(L, qk_bias)
    in_maps = [
        _prep_core_inputs(c, L, q, k, v, Wq, bq, Wk, bk, Wv, bv, Wo, bo)
        for c in range(N_CORES)
    ]
    res = run_bass_kernel_spmd(nc, in_maps, core_ids=list(range(N_CORES)))

    corr = (bv @ Wo.T + bo).astype(np.float32)  # folded-out V/O biases
    y = np.zeros((B, L, D), dtype=np.float32)
    cpb = N_CORES // B
    for c in range(N_CORES):
        y[c // cpb] += np.asarray(res.results[c]["y"], dtype=np.float32)
    y += corr[None, None, :]
    return y
